# revision 33
# baseline (speedup 1.0000x reference)
"""Trainium2 Bass kernel for causal multi-head attention with interleaved RoPE.

Problem: B=2, S=2048, D=1024, 16 heads x 64 dims, causal, rope theta=1e4.

Sharding (8 cores): 2-way batch x 4-way head tensor-parallel.
  core i: batch b = i // 4, head group g = i % 4 (heads 4g..4g+3, dims 256).
  Each core computes q/k/v for its heads from x[b], runs causal flash
  attention, and produces a partial output projection outT [D, S] (bf16).
  Host sums the 4 partials per batch and transposes.

Device design (v2 — pipelined, bf16):
  - All matmul operands are bf16 (1 PE cycle/row); PSUM accumulates f32.
  - One software-pipelined loop over 4 query tiles of 512:
    QKV projection + rope -> causal flash attention -> normalize ->
    output projection, emitted per-tile so the Tile scheduler overlaps
    phases across tiles and keeps the PE dense (HAM stays at 2.4 GHz).
  - Scores for the head pair (a=0,1) of a group go into one 2-bank PSUM
    tile [128, 1024] so a single EXP covers both heads (halves ACT
    overhead).  The softmax denominator comes from a ones-column
    appended to V (65-wide AV output).
  - Causality: only k-chunks up to the diagonal are processed; diagonal
    128x128 blocks are zeroed post-exp with a 0/1 mask on GpSimd (one
    two-window op covers both heads).
  - Normalization: sums rows are copied to SBUF, reciprocal'd with one
    DVE reciprocal_approx_fast, partition-broadcast with a single
    stride-0 SBUF->SBUF DMA, and multiplied into the copied oT tile.
"""

import os
import sys

sys.path.insert(0, "/opt/trn_rl_repo")

import numpy as np

B = 2
S = 2048
D = 1024
NH = 16
HD = 64
THETA = 10000.0
NCORES = 8
HPC = 4  # heads per core
DC = HPC * HD  # 256 dims per core
GQ = 2  # 128-partition groups per core for q/k/o dims (DC/128)
QT = 512  # query tile (free dim)
NQT = S // QT
KC = 128  # key chunk (partition dim)

_CACHE = {}


def _install_axon_ntff_hook():
    """Register antenv.axon_hooks so trace=True (BASS_TRACE=1) works."""
    import types

    if "antenv.axon_hooks" in sys.modules:
        return
    m = types.ModuleType("antenv.axon_hooks")
    _hook = [None]
    m.set_axon_ntff_profile_hook = lambda h: _hook.__setitem__(0, h)
    m.get_axon_ntff_profile_hook = lambda: _hook[0]
    sys.modules["antenv.axon_hooks"] = m
    try:
        import antenv

        antenv.axon_hooks = m
        from trn_agent_boot.trn_boot import _ntff_profile_via_ctypes

        hook = _ntff_profile_via_ctypes("/opt/axon/libaxon_pjrt.so")
        if hook is not None:
            m.set_axon_ntff_profile_hook(hook)
    except Exception:
        pass


def _rope_perm_local():
    """Permutation of one head's 64 dims: original interleaved pair (2i, 2i+1)
    -> t0 at quadrant*32 + (i%16), t1 at quadrant*32 + 16 + (i%16), with
    quadrant = i // 16.  Returns perm such that new[j] = old[perm[j]]."""
    perm = np.zeros(HD, dtype=np.int64)
    for i in range(HD // 2):
        qd, r = divmod(i, 16)
        perm[qd * 32 + r] = 2 * i
        perm[qd * 32 + 16 + r] = 2 * i + 1
    return perm


def _rope_tables():
    """cos_dup/sin_signed [128, S]: per-partition rope tables matching the
    de-interleaved layout (pattern repeats every 64 partitions)."""
    inv_freq = 1.0 / (THETA ** (np.arange(0, HD, 2, dtype=np.float64) / HD))  # [32]
    pos = np.arange(S, dtype=np.float64)
    ang = pos[None, :] * inv_freq[:, None]  # [32, S]
    cos = np.cos(ang)
    sin = np.sin(ang)
    cos_dup = np.zeros((128, S), dtype=np.float32)
    sin_signed = np.zeros((128, S), dtype=np.float32)
    for p in range(128):
        d = p % HD
        qd, r0 = divmod(d, 32)
        if r0 < 16:
            i = qd * 16 + r0
            cos_dup[p] = cos[i]
            sin_signed[p] = -sin[i]
        else:
            i = qd * 16 + (r0 - 16)
            cos_dup[p] = cos[i]
            sin_signed[p] = sin[i]
    return cos_dup, sin_signed


def _build_program():
    import concourse.bass as bass
    from concourse import bacc, mybir
    import concourse.tile as tile

    f32 = mybir.dt.float32
    bf16 = mybir.dt.bfloat16
    ADD = mybir.AluOpType.add
    MULT = mybir.AluOpType.mult
    EXP = mybir.ActivationFunctionType.Exp
    SWAP16 = [(j + 16) % 32 for j in range(32)]
    AP = bass.AP

    def win2(t, lo, hi, width=None):
        """Two-window AP over a [128, 2*QT] tile: free dims
        [[QT, 2], [1, hi-lo]] starting at column `lo` (covers columns
        [lo:hi] and [QT+lo:QT+hi])."""
        u = t[:, lo:QT]
        w = (hi - lo) if width is None else width
        return AP(tensor=u.tensor, offset=u.offset,
                  ap=[list(u.ap[0]), [QT, 2], [1, w]])

    nc = bacc.Bacc("TRN2", target_bir_lowering=False, debug=False)
    xT = nc.dram_tensor("xT", [D, S], bf16, kind="ExternalInput").ap()
    wq = nc.dram_tensor("wq", [D, DC], bf16, kind="ExternalInput").ap()
    wk = nc.dram_tensor("wk", [D, DC], bf16, kind="ExternalInput").ap()
    wv = nc.dram_tensor("wv", [D, DC], bf16, kind="ExternalInput").ap()
    wo = nc.dram_tensor("wo", [DC, D], bf16, kind="ExternalInput").ap()
    cosd = nc.dram_tensor("cosd", [128, S], f32, kind="ExternalInput").ap()
    sind = nc.dram_tensor("sind", [128, S], bf16, kind="ExternalInput").ap()
    trip = nc.dram_tensor("trip", [KC, 2 * KC], bf16, kind="ExternalInput").ap()
    outT = nc.dram_tensor("outT", [D, S], bf16, kind="ExternalOutput").ap()
    debug = os.environ.get("BASS_DEBUG_DUMP")
    if debug:
        qT_d = nc.dram_tensor("qT_d", [128, GQ, S], bf16, kind="ExternalOutput").ap()
        kT_d = nc.dram_tensor("kT_d", [128, GQ, S], bf16, kind="ExternalOutput").ap()
        va_d = nc.dram_tensor("va_d", [128, S // KC, HPC * (HD + 1)], bf16,
                              kind="ExternalOutput").ap()
        oT_d = nc.dram_tensor("oT_d", [128, GQ, S], bf16, kind="ExternalOutput").ap()
        pr_d = nc.dram_tensor("pr_d", [128, 2 * QT], bf16, kind="ExternalOutput").ap()
        rb_d = nc.dram_tensor("rb_d", [HD, QT], f32, kind="ExternalOutput").ap()

    with tile.TileContext(nc) as tc:
        with tc.tile_pool(name="const", bufs=1) as const, \
             tc.tile_pool(name="ps_misc", bufs=2, space="PSUM") as ps_misc, \
             tc.tile_pool(name="ps_s", bufs=2, space="PSUM") as ps_s_pool, \
             tc.tile_pool(name="ps_o", bufs=1, space="PSUM") as ps_o_pool, \
             tc.tile_pool(name="probs", bufs=6) as probs_pool, \
             tc.tile_pool(name="rope", bufs=3) as rope_pool, \
             tc.tile_pool(name="ob", bufs=3) as ob_pool, \
             tc.tile_pool(name="norm", bufs=3) as norm_pool:

            # ---- constants / persistent tensors ----
            cos_sb = const.tile([128, S], f32)
            sin_sb = const.tile([128, S], bf16)
            trip_sb = const.tile([KC, 2 * KC], bf16)
            wq_sb = const.tile([128, D // 128, DC], bf16)
            wk_sb = const.tile([128, D // 128, DC], bf16)
            wv_sb = const.tile([128, D // 128, DC], bf16)
            wo_sb = const.tile([128, GQ, D], bf16)
            xT_sb = const.tile([128, D // 128, S], bf16)
            qT_sb = const.tile([128, GQ, S], bf16)
            kT_sb = const.tile([128, GQ, S], bf16)
            vaug_sb = const.tile([128, S // KC, HPC * (HD + 1)], bf16)
            oT_sb = const.tile([128, GQ, S], bf16)

            nc.sync.dma_start(wq_sb, wq.rearrange("(o p) n -> p o n", p=128))
            nc.sync.dma_start(wk_sb, wk.rearrange("(o p) n -> p o n", p=128))
            nc.scalar.dma_start(cos_sb, cosd)
            nc.scalar.dma_start(sin_sb, sind)
            nc.gpsimd.dma_start(wv_sb, wv.rearrange("(o p) n -> p o n", p=128))
            nc.gpsimd.dma_start(trip_sb, trip)
            nc.gpsimd.dma_start(wo_sb, wo.rearrange("(o p) n -> p o n", p=128))
            # ones column of v_aug (slot 64 of each head's 65-wide block)
            nc.gpsimd.memset(vaug_sb[:, :, HD::(HD + 1)], 1.0)

            xT_dram = xT.rearrange("(o p) n -> p o n", p=128)

            def rope(ps, dst, q0):
                shuf = rope_pool.tile([128, QT], f32, tag="shuf")
                nc.vector.stream_shuffle(shuf, ps, SWAP16)
                m1 = rope_pool.tile([128, QT], bf16, tag="m1")
                nc.vector.tensor_tensor(m1, ps, cos_sb[:, q0:q0 + QT], MULT)
                m2 = rope_pool.tile([128, QT], bf16, tag="m2")
                nc.gpsimd.tensor_tensor(m2, shuf, sin_sb[:, q0:q0 + QT], MULT)
                nc.gpsimd.tensor_tensor(dst, m1, m2, ADD)

            def xt_load(qt):
                q0 = qt * QT
                # load this tile's x columns (chunked so the first matmul
                # can start as soon as chunk 0 lands)
                for kc in range(D // 128):
                    nc.sync.dma_start(xT_sb[:, kc, q0:q0 + QT],
                                      xT_dram[:, kc, q0:q0 + QT])

            def qk_pair(qt, g):
                q0 = qt * QT
                ps_q = ps_misc.tile([128, QT], f32, tag="mm", name="ps_q")
                for kc in range(D // 128):
                    nc.tensor.matmul(
                        ps_q, wq_sb[:, kc, g * 128:(g + 1) * 128],
                        xT_sb[:, kc, q0:q0 + QT],
                        start=(kc == 0), stop=(kc == D // 128 - 1))
                rope(ps_q, qT_sb[:, g, q0:q0 + QT], q0)
                ps_k = ps_misc.tile([128, QT], f32, tag="mm", name="ps_k")
                for kc in range(D // 128):
                    nc.tensor.matmul(
                        ps_k, wk_sb[:, kc, g * 128:(g + 1) * 128],
                        xT_sb[:, kc, q0:q0 + QT],
                        start=(kc == 0), stop=(kc == D // 128 - 1))
                rope(ps_k, kT_sb[:, g, q0:q0 + QT], q0)

            def v_chunk(rc):
                # V projection for one 128-row seq chunk
                ps_v = ps_s_pool.tile([128, 2 * QT], f32, tag="s",
                                      name="ps_v")
                for kc in range(D // 128):
                    nc.tensor.matmul(
                        ps_v[:, 0:DC],
                        xT_sb[:, kc, rc * KC:(rc + 1) * KC],
                        wv_sb[:, kc, :],
                        start=(kc == 0), stop=(kc == D // 128 - 1))
                # strided copy into the 65-wide head slots
                u = vaug_sb[:, rc, 0:HPC * (HD + 1)]
                dst = AP(tensor=u.tensor, offset=u.offset,
                         ap=[list(u.ap[0]), [HD + 1, HPC], [1, HD]])
                v = ps_v[:, 0:DC]
                src = AP(tensor=v.tensor, offset=v.offset,
                         ap=[list(v.ap[0]), [HD, HPC], [1, HD]])
                nc.vector.tensor_copy(out=dst, in_=src)

            def att_group(qt, g, fillers=()):
                """Causal flash attention for (tile qt, group g) with the AV
                matmul software-lagged one k-chunk behind the scores so the
                exp latency never blocks the PE.  `fillers` are independent
                emission thunks sprinkled into the kc loop so their pool-slot
                requests interleave with the scores stream."""
                q0 = qt * QT
                nkc = (q0 + QT) // KC
                fillers = list(fillers)
                fill_at = {(1 + i) * nkc // (len(fillers) + 1): i
                           for i in range(len(fillers))} if fillers else {}
                ps_o = [ps_o_pool.tile([HD + 1, QT], f32, tag=f"o{a}",
                                       name=f"ps_o{a}")
                        for a in range(2)]
                pend = None  # (kc, qlo, probs) awaiting its AV matmuls

                def av(kc, qlo, probs):
                    for a in range(2):
                        h = 2 * g + a
                        nc.tensor.matmul(
                            ps_o[a][:, qlo:QT],
                            vaug_sb[:, kc, h * (HD + 1):(h + 1) * (HD + 1)],
                            probs[:, a * QT + qlo:(a + 1) * QT],
                            start=(kc == 0), stop=(kc == nkc - 1))

                for kc in range(nkc):
                    k0 = kc * KC
                    qlo = max(0, k0 - q0)
                    ps_s = ps_s_pool.tile([128, 2 * QT], f32, tag="s",
                                          name="ps_s")
                    for a in range(2):
                        nc.tensor.matmul(
                            ps_s[:, a * QT + qlo:(a + 1) * QT],
                            kT_sb[a * HD:(a + 1) * HD, g, k0:k0 + KC],
                            qT_sb[a * HD:(a + 1) * HD, g, q0 + qlo:q0 + QT],
                            start=True, stop=True)
                    probs = probs_pool.tile([128, 2 * QT], bf16, tag="p")
                    nc.scalar.activation(
                        win2(probs, qlo, QT), win2(ps_s, qlo, QT), EXP)
                    if k0 >= q0:
                        # zero the strictly-upper part of the diag blocks
                        nc.vector.tensor_tensor(
                            win2(probs, qlo, qlo + KC),
                            win2(probs, qlo, qlo + KC),
                            AP(tensor=trip_sb.tensor, offset=trip_sb.offset,
                               ap=[list(trip_sb.ap[0]), [KC, 2], [1, KC]]),
                            MULT)
                    if debug and qt == 0 and g == 0 and kc == 0:
                        nc.sync.dma_start(pr_d, probs)
                    if pend is not None:
                        av(*pend)
                    pend = (kc, qlo, probs)
                    if kc in fill_at:
                        fillers[fill_at[kc]]()
                av(*pend)

                # ---- normalize: copy out of PSUM fast (releases the AV
                # accumulator bank), then recip/bcast/mult in SBUF ----
                for a in range(2):
                    oraw = norm_pool.tile([HD + 1, QT], f32, tag="oraw",
                                          name="oraw")
                    nc.vector.tensor_copy(out=oraw, in_=ps_o[a])
                    sraw = norm_pool.tile([1, QT], f32, tag="sraw",
                                          name="sraw")
                    nc.vector.tensor_copy(out=sraw, in_=ps_o[a][HD:HD + 1, :])
                    srow = norm_pool.tile([1, QT], f32, tag="srow",
                                          name="srow")
                    nc.vector.reciprocal_approx_fast(srow, sraw)
                    rbc = norm_pool.tile([HD, QT], f32, tag="rbc",
                                         name="rbc")
                    nc.gpsimd.partition_broadcast(rbc, srow)
                    if debug and qt == 0 and g == 0 and a == 0:
                        nc.sync.dma_start(rb_d, rbc)
                    nc.gpsimd.tensor_tensor(
                        oT_sb[a * HD:(a + 1) * HD, g, q0:q0 + QT],
                        oraw[0:HD, :], rbc, MULT)

            def proj_tile(qt):
                q0 = qt * QT
                for ec in range(D // 128):
                    ps = ps_misc.tile([128, QT], f32, tag="mm", name="ps_pr")
                    for g in range(GQ):
                        nc.tensor.matmul(
                            ps, wo_sb[:, g, ec * 128:(ec + 1) * 128],
                            oT_sb[:, g, q0:q0 + QT],
                            start=(g == 0), stop=(g == GQ - 1))
                    ob = ob_pool.tile([128, QT], bf16, tag="ob")
                    nc.vector.tensor_copy(out=ob, in_=ps)
                    nc.sync.dma_start(outT[ec * 128:(ec + 1) * 128, q0:q0 + QT],
                                      ob)

            xt_load(0)
            qk_pair(0, 0)
            qk_pair(0, 1)
            for rc in range(4):
                v_chunk(rc)
            pending_proj = None
            for qt in range(NQT):
                if qt + 1 < NQT:
                    xt_load(qt + 1)
                att_group(qt, 0)
                if pending_proj is not None:
                    proj_tile(pending_proj)
                    pending_proj = None
                if qt + 1 < NQT:
                    qk_pair(qt + 1, 0)
                if qt + 1 < NQT:
                    vs = [(lambda rc=rc: v_chunk(rc))
                          for rc in range(4 * (qt + 1), 4 * (qt + 1) + 4)]
                    att_group(qt, 1, fillers=vs)
                    qk_pair(qt + 1, 1)
                else:
                    att_group(qt, 1)
                pending_proj = qt
            proj_tile(pending_proj)

            if debug:
                nc.sync.dma_start(qT_d, qT_sb)
                nc.sync.dma_start(kT_d, kT_sb)
                nc.sync.dma_start(va_d, vaug_sb)
                nc.sync.dma_start(oT_d, oT_sb)

    nc.finalize()
    return nc


def kernel(x, wq, wk, wv, wo):
    import ml_dtypes
    from concourse import bass_utils

    if os.environ.get("BASS_TRACE"):
        _install_axon_ntff_hook()

    bf = ml_dtypes.bfloat16
    x = np.asarray(x, dtype=np.float32)
    wq = np.asarray(wq, dtype=np.float32)
    wk = np.asarray(wk, dtype=np.float32)
    wv = np.asarray(wv, dtype=np.float32)
    wo = np.asarray(wo, dtype=np.float32)

    # Host prep: weight slicing + rope column permutation + tables.
    perm_l = _rope_perm_local()
    perm = np.concatenate([h * HD + perm_l for h in range(NH)])  # [D]
    scale = 1.0 / np.sqrt(HD)
    wq_p = (wq[:, perm] * scale).astype(bf)
    wk_p = wk[:, perm].astype(bf)
    wv_b = wv.astype(bf)
    wo_b = wo.astype(bf)
    cos_dup, sin_signed = _rope_tables()
    sin_b = sin_signed.astype(bf)
    kl = np.arange(KC)[:, None]
    ql = np.arange(KC)[None, :]
    tri01 = (ql >= kl).astype(bf)
    trip = np.ascontiguousarray(np.concatenate([tri01, tri01], axis=1))

    xTs = [np.ascontiguousarray(x[b].T).astype(bf) for b in range(B)]

    in_maps = []
    for i in range(NCORES):
        b, g = divmod(i, HPC)
        cs = slice(g * DC, (g + 1) * DC)
        in_maps.append({
            "xT": xTs[b],
            "wq": np.ascontiguousarray(wq_p[:, cs]),
            "wk": np.ascontiguousarray(wk_p[:, cs]),
            "wv": np.ascontiguousarray(wv_b[:, cs]),
            "wo": np.ascontiguousarray(wo_b[cs, :]),
            "cosd": cos_dup,
            "sind": sin_b,
            "trip": trip,
        })

    if "nc" not in _CACHE:
        _CACHE["nc"] = _build_program()
    nc = _CACHE["nc"]

    res = bass_utils.run_bass_kernel_spmd(nc, in_maps, core_ids=list(range(NCORES)))
    _CACHE["last_exec_time_ns"] = res.exec_time_ns
    _CACHE["last_res"] = res

    out = np.empty((B, S, D), dtype=np.float32)
    for b in range(B):
        acc = res.results[b * HPC]["outT"].astype(np.float32)
        for g in range(1, HPC):
            acc += res.results[b * HPC + g]["outT"].astype(np.float32)
        out[b] = acc.T
    return out


# revision 34
# speedup vs baseline: 1.6536x; 1.6536x over previous
"""Trainium2 Bass kernel for causal multi-head attention with interleaved RoPE.

Problem: B=2, S=2048, D=1024, 16 heads x 64 dims, causal, rope theta=1e4.

Sharding (8 cores): 2-way batch x 4-way head tensor-parallel.
  core i: batch b = i // 4, head group g = i % 4 (heads 4g..4g+3, dims 256).
  Each core computes q/k/v for its heads from x[b], runs causal flash
  attention, and produces a partial output projection outT [D, S] (bf16).
  Host sums the 4 partials per batch and transposes.

Device design (v2 — pipelined, bf16):
  - All matmul operands are bf16 (1 PE cycle/row); PSUM accumulates f32.
  - One software-pipelined loop over 4 query tiles of 512:
    QKV projection + rope -> causal flash attention -> normalize ->
    output projection, emitted per-tile so the Tile scheduler overlaps
    phases across tiles and keeps the PE dense (HAM stays at 2.4 GHz).
  - Scores for the head pair (a=0,1) of a group go into one 2-bank PSUM
    tile [128, 1024] so a single EXP covers both heads (halves ACT
    overhead).  The softmax denominator comes from a ones-column
    appended to V (65-wide AV output).
  - Causality: only k-chunks up to the diagonal are processed; diagonal
    128x128 blocks are zeroed post-exp with a 0/1 mask on GpSimd (one
    two-window op covers both heads).
  - Normalization: sums rows are copied to SBUF, reciprocal'd with one
    DVE reciprocal_approx_fast, partition-broadcast with a single
    stride-0 SBUF->SBUF DMA, and multiplied into the copied oT tile.
"""

import os
import sys

sys.path.insert(0, "/opt/trn_rl_repo")

import numpy as np

B = 2
S = 2048
D = 1024
NH = 16
HD = 64
THETA = 10000.0
NCORES = 8
HPC = 4  # heads per core
DC = HPC * HD  # 256 dims per core
GQ = 2  # 128-partition groups per core for q/k/o dims (DC/128)
QT = 512  # query tile (free dim)
NQT = S // QT
KC = 128  # key chunk (partition dim)

_CACHE = {}


def _install_axon_ntff_hook():
    """Register antenv.axon_hooks so trace=True (BASS_TRACE=1) works."""
    import types

    if "antenv.axon_hooks" in sys.modules:
        return
    m = types.ModuleType("antenv.axon_hooks")
    _hook = [None]
    m.set_axon_ntff_profile_hook = lambda h: _hook.__setitem__(0, h)
    m.get_axon_ntff_profile_hook = lambda: _hook[0]
    sys.modules["antenv.axon_hooks"] = m
    try:
        import antenv

        antenv.axon_hooks = m
        from trn_agent_boot.trn_boot import _ntff_profile_via_ctypes

        hook = _ntff_profile_via_ctypes("/opt/axon/libaxon_pjrt.so")
        if hook is not None:
            m.set_axon_ntff_profile_hook(hook)
    except Exception:
        pass


def _rope_perm_local():
    """Permutation of one head's 64 dims: original interleaved pair (2i, 2i+1)
    -> t0 at quadrant*32 + (i%16), t1 at quadrant*32 + 16 + (i%16), with
    quadrant = i // 16.  Returns perm such that new[j] = old[perm[j]]."""
    perm = np.zeros(HD, dtype=np.int64)
    for i in range(HD // 2):
        qd, r = divmod(i, 16)
        perm[qd * 32 + r] = 2 * i
        perm[qd * 32 + 16 + r] = 2 * i + 1
    return perm


def _rope_tables():
    """cos_dup/sin_signed [128, S]: per-partition rope tables matching the
    de-interleaved layout (pattern repeats every 64 partitions)."""
    inv_freq = 1.0 / (THETA ** (np.arange(0, HD, 2, dtype=np.float64) / HD))  # [32]
    pos = np.arange(S, dtype=np.float64)
    ang = pos[None, :] * inv_freq[:, None]  # [32, S]
    cos = np.cos(ang)
    sin = np.sin(ang)
    cos_dup = np.zeros((128, S), dtype=np.float32)
    sin_signed = np.zeros((128, S), dtype=np.float32)
    for p in range(128):
        d = p % HD
        qd, r0 = divmod(d, 32)
        if r0 < 16:
            i = qd * 16 + r0
            cos_dup[p] = cos[i]
            sin_signed[p] = -sin[i]
        else:
            i = qd * 16 + (r0 - 16)
            cos_dup[p] = cos[i]
            sin_signed[p] = sin[i]
    return cos_dup, sin_signed


def _build_program():
    import concourse.bass as bass
    from concourse import bacc, mybir
    import concourse.tile as tile

    f32 = mybir.dt.float32
    bf16 = mybir.dt.bfloat16
    ADD = mybir.AluOpType.add
    MULT = mybir.AluOpType.mult
    EXP = mybir.ActivationFunctionType.Exp
    SWAP16 = [(j + 16) % 32 for j in range(32)]
    AP = bass.AP

    def win2(t, lo, hi, width=None):
        """Two-window AP over a [128, 2*QT] tile: free dims
        [[QT, 2], [1, hi-lo]] starting at column `lo` (covers columns
        [lo:hi] and [QT+lo:QT+hi])."""
        u = t[:, lo:QT]
        w = (hi - lo) if width is None else width
        return AP(tensor=u.tensor, offset=u.offset,
                  ap=[list(u.ap[0]), [QT, 2], [1, w]])

    nc = bacc.Bacc("TRN2", target_bir_lowering=False, debug=False)
    xT = nc.dram_tensor("xT", [D, S], bf16, kind="ExternalInput").ap()
    wq = nc.dram_tensor("wq", [D, DC], bf16, kind="ExternalInput").ap()
    wk = nc.dram_tensor("wk", [D, DC], bf16, kind="ExternalInput").ap()
    wv = nc.dram_tensor("wv", [D, DC], bf16, kind="ExternalInput").ap()
    wo = nc.dram_tensor("wo", [DC, D], bf16, kind="ExternalInput").ap()
    cosd = nc.dram_tensor("cosd", [128, S], f32, kind="ExternalInput").ap()
    sind = nc.dram_tensor("sind", [128, S], bf16, kind="ExternalInput").ap()
    trip = nc.dram_tensor("trip", [KC, 2 * KC], bf16, kind="ExternalInput").ap()
    outT = nc.dram_tensor("outT", [D, S], bf16, kind="ExternalOutput").ap()
    debug = os.environ.get("BASS_DEBUG_DUMP")
    if debug:
        qT_d = nc.dram_tensor("qT_d", [128, GQ, S], bf16, kind="ExternalOutput").ap()
        kT_d = nc.dram_tensor("kT_d", [128, GQ, S], bf16, kind="ExternalOutput").ap()
        va_d = nc.dram_tensor("va_d", [128, S // KC, HPC * (HD + 1)], bf16,
                              kind="ExternalOutput").ap()
        oT_d = nc.dram_tensor("oT_d", [128, GQ, S], bf16, kind="ExternalOutput").ap()
        pr_d = nc.dram_tensor("pr_d", [128, 2 * QT], bf16, kind="ExternalOutput").ap()
        rb_d = nc.dram_tensor("rb_d", [HD, QT], f32, kind="ExternalOutput").ap()

    with tile.TileContext(nc) as tc:
        with tc.tile_pool(name="const", bufs=1) as const, \
             tc.tile_pool(name="ps_misc", bufs=2, space="PSUM") as ps_misc, \
             tc.tile_pool(name="ps_s", bufs=2, space="PSUM") as ps_s_pool, \
             tc.tile_pool(name="ps_o", bufs=1, space="PSUM") as ps_o_pool, \
             tc.tile_pool(name="probs", bufs=6) as probs_pool, \
             tc.tile_pool(name="rope", bufs=3) as rope_pool, \
             tc.tile_pool(name="ob", bufs=3) as ob_pool, \
             tc.tile_pool(name="norm", bufs=3) as norm_pool:

            # ---- constants / persistent tensors ----
            cos_sb = const.tile([128, S], f32)
            sin_sb = const.tile([128, S], bf16)
            trip_sb = const.tile([KC, 2 * KC], bf16)
            wq_sb = const.tile([128, D // 128, DC], bf16)
            wk_sb = const.tile([128, D // 128, DC], bf16)
            wv_sb = const.tile([128, D // 128, DC], bf16)
            wo_sb = const.tile([128, GQ, D], bf16)
            xT_sb = const.tile([128, D // 128, S], bf16)
            qT_sb = const.tile([128, GQ, S], bf16)
            kT_sb = const.tile([128, GQ, S], bf16)
            vaug_sb = const.tile([128, S // KC, HPC * (HD + 1)], bf16)
            oT_sb = const.tile([128, GQ, S], bf16)

            nc.sync.dma_start(wq_sb, wq.rearrange("(o p) n -> p o n", p=128))
            nc.sync.dma_start(wk_sb, wk.rearrange("(o p) n -> p o n", p=128))
            nc.scalar.dma_start(cos_sb, cosd)
            nc.scalar.dma_start(sin_sb, sind)
            nc.gpsimd.dma_start(wv_sb, wv.rearrange("(o p) n -> p o n", p=128))
            nc.gpsimd.dma_start(trip_sb, trip)
            nc.gpsimd.dma_start(wo_sb, wo.rearrange("(o p) n -> p o n", p=128))
            # ones column of v_aug (slot 64 of each head's 65-wide block)
            nc.gpsimd.memset(vaug_sb[:, :, HD::(HD + 1)], 1.0)

            xT_dram = xT.rearrange("(o p) n -> p o n", p=128)

            def rope(ps, dst, q0):
                shuf = rope_pool.tile([128, QT], f32, tag="shuf")
                nc.vector.stream_shuffle(shuf, ps, SWAP16)
                m1 = rope_pool.tile([128, QT], bf16, tag="m1")
                nc.vector.tensor_tensor(m1, ps, cos_sb[:, q0:q0 + QT], MULT)
                m2 = rope_pool.tile([128, QT], bf16, tag="m2")
                nc.gpsimd.tensor_tensor(m2, shuf, sin_sb[:, q0:q0 + QT], MULT)
                nc.gpsimd.tensor_tensor(dst, m1, m2, ADD)

            def xt_load(qt):
                q0 = qt * QT
                # load this tile's x columns (chunked so the first matmul
                # can start as soon as chunk 0 lands)
                for kc in range(D // 128):
                    nc.sync.dma_start(xT_sb[:, kc, q0:q0 + QT],
                                      xT_dram[:, kc, q0:q0 + QT])

            def qk_pair(qt, g):
                q0 = qt * QT
                ps_q = ps_misc.tile([128, QT], f32, tag="mm", name="ps_q")
                for kc in range(D // 128):
                    nc.tensor.matmul(
                        ps_q, wq_sb[:, kc, g * 128:(g + 1) * 128],
                        xT_sb[:, kc, q0:q0 + QT],
                        start=(kc == 0), stop=(kc == D // 128 - 1))
                rope(ps_q, qT_sb[:, g, q0:q0 + QT], q0)
                ps_k = ps_misc.tile([128, QT], f32, tag="mm", name="ps_k")
                for kc in range(D // 128):
                    nc.tensor.matmul(
                        ps_k, wk_sb[:, kc, g * 128:(g + 1) * 128],
                        xT_sb[:, kc, q0:q0 + QT],
                        start=(kc == 0), stop=(kc == D // 128 - 1))
                rope(ps_k, kT_sb[:, g, q0:q0 + QT], q0)

            def v_chunk(rc):
                # V projection for one 128-row seq chunk
                ps_v = ps_s_pool.tile([128, 2 * QT], f32, tag="s",
                                      name="ps_v")
                for kc in range(D // 128):
                    nc.tensor.matmul(
                        ps_v[:, 0:DC],
                        xT_sb[:, kc, rc * KC:(rc + 1) * KC],
                        wv_sb[:, kc, :],
                        start=(kc == 0), stop=(kc == D // 128 - 1))
                # strided copy into the 65-wide head slots
                u = vaug_sb[:, rc, 0:HPC * (HD + 1)]
                dst = AP(tensor=u.tensor, offset=u.offset,
                         ap=[list(u.ap[0]), [HD + 1, HPC], [1, HD]])
                v = ps_v[:, 0:DC]
                src = AP(tensor=v.tensor, offset=v.offset,
                         ap=[list(v.ap[0]), [HD, HPC], [1, HD]])
                nc.vector.tensor_copy(out=dst, in_=src)

            def att_group(qt, g, fillers=()):
                """Causal flash attention for (tile qt, group g) with the AV
                matmul software-lagged one k-chunk behind the scores so the
                exp latency never blocks the PE.  `fillers` are independent
                emission thunks sprinkled into the kc loop so their pool-slot
                requests interleave with the scores stream."""
                q0 = qt * QT
                nkc = (q0 + QT) // KC
                fillers = list(fillers)
                fill_at = {(1 + i) * nkc // (len(fillers) + 1): i
                           for i in range(len(fillers))} if fillers else {}
                ps_o = [ps_o_pool.tile([HD + 1, QT], f32, tag=f"o{a}",
                                       name=f"ps_o{a}")
                        for a in range(2)]
                pend = None  # (kc, qlo, probs) awaiting its AV matmuls

                def av(kc, qlo, probs):
                    for a in range(2):
                        h = 2 * g + a
                        nc.tensor.matmul(
                            ps_o[a][:, qlo:QT],
                            vaug_sb[:, kc, h * (HD + 1):(h + 1) * (HD + 1)],
                            probs[:, a * QT + qlo:(a + 1) * QT],
                            start=(kc == 0), stop=(kc == nkc - 1))

                for kc in range(nkc):
                    k0 = kc * KC
                    qlo = max(0, k0 - q0)
                    ps_s = ps_s_pool.tile([128, 2 * QT], f32, tag="s",
                                          name="ps_s")
                    for a in range(2):
                        nc.tensor.matmul(
                            ps_s[:, a * QT + qlo:(a + 1) * QT],
                            kT_sb[a * HD:(a + 1) * HD, g, k0:k0 + KC],
                            qT_sb[a * HD:(a + 1) * HD, g, q0 + qlo:q0 + QT],
                            start=True, stop=True)
                    probs = probs_pool.tile([128, 2 * QT], bf16, tag="p")
                    nc.scalar.activation(
                        win2(probs, qlo, QT), win2(ps_s, qlo, QT), EXP)
                    if k0 >= q0:
                        # zero the strictly-upper part of the diag blocks
                        nc.vector.tensor_tensor(
                            win2(probs, qlo, qlo + KC),
                            win2(probs, qlo, qlo + KC),
                            AP(tensor=trip_sb.tensor, offset=trip_sb.offset,
                               ap=[list(trip_sb.ap[0]), [KC, 2], [1, KC]]),
                            MULT)
                    if debug and qt == 0 and g == 0 and kc == 0:
                        nc.sync.dma_start(pr_d, probs)
                    if pend is not None:
                        av(*pend)
                    pend = (kc, qlo, probs)
                    if kc in fill_at:
                        fillers[fill_at[kc]]()
                av(*pend)

                # ---- normalize: copy out of PSUM fast (releases the AV
                # accumulator bank), then recip/bcast/mult in SBUF ----
                for a in range(2):
                    oraw = norm_pool.tile([HD + 1, QT], f32, tag="oraw",
                                          name="oraw")
                    nc.vector.tensor_copy(out=oraw, in_=ps_o[a])
                    sraw = norm_pool.tile([1, QT], f32, tag="sraw",
                                          name="sraw")
                    nc.vector.tensor_copy(out=sraw, in_=ps_o[a][HD:HD + 1, :])
                    srow = norm_pool.tile([1, QT], f32, tag="srow",
                                          name="srow")
                    nc.vector.reciprocal_approx_fast(srow, sraw)
                    rbc = norm_pool.tile([HD, QT], f32, tag="rbc",
                                         name="rbc")
                    nc.gpsimd.partition_broadcast(rbc, srow)
                    if debug and qt == 0 and g == 0 and a == 0:
                        nc.sync.dma_start(rb_d, rbc)
                    nc.vector.tensor_tensor(
                        oT_sb[a * HD:(a + 1) * HD, g, q0:q0 + QT],
                        oraw[0:HD, :], rbc, MULT)

            def proj_tile(qt):
                q0 = qt * QT
                for ec in range(D // 128):
                    ps = ps_misc.tile([128, QT], f32, tag="mm", name="ps_pr")
                    for g in range(GQ):
                        nc.tensor.matmul(
                            ps, wo_sb[:, g, ec * 128:(ec + 1) * 128],
                            oT_sb[:, g, q0:q0 + QT],
                            start=(g == 0), stop=(g == GQ - 1))
                    ob = ob_pool.tile([128, QT], bf16, tag="ob")
                    nc.vector.tensor_copy(out=ob, in_=ps)
                    nc.sync.dma_start(outT[ec * 128:(ec + 1) * 128, q0:q0 + QT],
                                      ob)

            xt_load(0)
            qk_pair(0, 0)
            qk_pair(0, 1)
            for rc in range(4):
                v_chunk(rc)
            pending_proj = None
            for qt in range(NQT):
                if qt + 1 < NQT:
                    xt_load(qt + 1)
                att_group(qt, 0)
                if pending_proj is not None:
                    proj_tile(pending_proj)
                    pending_proj = None
                if qt + 1 < NQT:
                    qk_pair(qt + 1, 0)
                if qt + 1 < NQT:
                    vs = [(lambda rc=rc: v_chunk(rc))
                          for rc in range(4 * (qt + 1), 4 * (qt + 1) + 4)]
                    att_group(qt, 1, fillers=vs)
                    qk_pair(qt + 1, 1)
                else:
                    att_group(qt, 1)
                pending_proj = qt
            proj_tile(pending_proj)

            if debug:
                nc.sync.dma_start(qT_d, qT_sb)
                nc.sync.dma_start(kT_d, kT_sb)
                nc.sync.dma_start(va_d, vaug_sb)
                nc.sync.dma_start(oT_d, oT_sb)

    nc.finalize()
    return nc


def kernel(x, wq, wk, wv, wo):
    import ml_dtypes
    from concourse import bass_utils

    if os.environ.get("BASS_TRACE"):
        _install_axon_ntff_hook()

    bf = ml_dtypes.bfloat16
    x = np.asarray(x, dtype=np.float32)
    wq = np.asarray(wq, dtype=np.float32)
    wk = np.asarray(wk, dtype=np.float32)
    wv = np.asarray(wv, dtype=np.float32)
    wo = np.asarray(wo, dtype=np.float32)

    # Host prep: weight slicing + rope column permutation + tables.
    perm_l = _rope_perm_local()
    perm = np.concatenate([h * HD + perm_l for h in range(NH)])  # [D]
    scale = 1.0 / np.sqrt(HD)
    wq_p = (wq[:, perm] * scale).astype(bf)
    wk_p = wk[:, perm].astype(bf)
    wv_b = wv.astype(bf)
    wo_b = wo.astype(bf)
    cos_dup, sin_signed = _rope_tables()
    sin_b = sin_signed.astype(bf)
    kl = np.arange(KC)[:, None]
    ql = np.arange(KC)[None, :]
    tri01 = (ql >= kl).astype(bf)
    trip = np.ascontiguousarray(np.concatenate([tri01, tri01], axis=1))

    xTs = [np.ascontiguousarray(x[b].T).astype(bf) for b in range(B)]

    in_maps = []
    for i in range(NCORES):
        b, g = divmod(i, HPC)
        cs = slice(g * DC, (g + 1) * DC)
        in_maps.append({
            "xT": xTs[b],
            "wq": np.ascontiguousarray(wq_p[:, cs]),
            "wk": np.ascontiguousarray(wk_p[:, cs]),
            "wv": np.ascontiguousarray(wv_b[:, cs]),
            "wo": np.ascontiguousarray(wo_b[cs, :]),
            "cosd": cos_dup,
            "sind": sin_b,
            "trip": trip,
        })

    if "nc" not in _CACHE:
        _CACHE["nc"] = _build_program()
    nc = _CACHE["nc"]

    res = bass_utils.run_bass_kernel_spmd(nc, in_maps, core_ids=list(range(NCORES)))
    _CACHE["last_exec_time_ns"] = res.exec_time_ns
    _CACHE["last_res"] = res

    out = np.empty((B, S, D), dtype=np.float32)
    for b in range(B):
        acc = res.results[b * HPC]["outT"].astype(np.float32)
        for g in range(1, HPC):
            acc += res.results[b * HPC + g]["outT"].astype(np.float32)
        out[b] = acc.T
    return out
